# revision 9
# baseline (speedup 1.0000x reference)
"""Distributed causal self-attention kernel for 8 TRN2 NeuronCores.

Problem: B=4, T=2048, C=1024, H=16 heads (hs=64), fp32.
  qkv = x @ W_attn + b_attn ; causal softmax attention ; out = y @ W_proj + b_proj

Sharding (Megatron, head-parallel):
  - Each core owns 2 heads (128 of the 1024 C-features).
  - Column-parallel QKV: core c gets W_attn columns for its heads -> [1024, 384].
  - Attention computed fully locally per (batch, head).
  - y^T (local 128 features x 8192 tokens) AllGathered -> Y^T [1024, 8192].
  - Column-parallel proj: core c computes out^T rows [128c:128c+128] (its out-column
    slice) over ALL tokens -> identical SPMD graph, no dynamic addressing.
  - Host concatenates the 8 out^T slices and transposes back.

Layout tricks (avoid all on-device transposes):
  - Host supplies xT = x^T [1024, 8192] so both the feature-major q/k matmuls
    (lhsT=W_qk, rhs=xT) and the token-major v matmul (lhsT=xT, rhs=W_v) need no
    transposes.
  - Attention computes S^T [keys, queries] = matmul(lhsT=kT, rhs=qT); softmax
    normalizer comes free by augmenting V with a ones column (row 64 of the AV
    PSUM accumulates sum_k P). exp has no max-subtraction (logits are O(5)).
  - Causal mask is multiplicative AFTER exp, only on diagonal 4-block groups.
"""

import os
import numpy as np

import concourse.bass as bass
import concourse.mybir as mybir
import concourse.tile as tile
from concourse import bacc
from concourse import bass_utils

F32 = mybir.dt.float32
F32R = mybir.dt.float32r

B, T, C = 4, 2048, 1024
NH, HS = 16, 64
NCORES = 8
HPC = NH // NCORES          # heads per core = 2
LC = HPC * HS               # local C features per core = 128
NTOK = B * T                # 8192
P = 128
KO = C // P                 # 8 contraction chunks
QC_W = 512                  # query-chunk width (PSUM bank)
KC_W = 128                  # key-chunk width (PSUM partition)
N_QC = T // QC_W            # 4 query chunks per batch
N_KC = T // KC_W            # 16 key chunks per batch
GRP = 4                     # key chunks per exp group (4*128 rows -> [128,4,512] psum)


def build_graph():
    nc = bacc.Bacc(
        "TRN2",
        target_bir_lowering=False,
        debug=False,
        enable_asserts=True,
        num_devices=NCORES,
    )

    xT = nc.dram_tensor("xT", [C, NTOK], F32R, kind="ExternalInput").ap()
    w_qkv = nc.dram_tensor("w_qkv", [C, 3 * LC], F32R, kind="ExternalInput").ap()
    b_qkv = nc.dram_tensor("b_qkv", [3 * LC], F32, kind="ExternalInput").ap()
    w_proj = nc.dram_tensor("w_proj", [C, LC], F32R, kind="ExternalInput").ap()
    b_proj = nc.dram_tensor("b_proj", [LC], F32, kind="ExternalInput").ap()
    out = nc.dram_tensor("out", [LC, NTOK], F32, kind="ExternalOutput").ap()

    xT_t = xT.rearrange("(ko p) t -> p ko t", p=P)         # [128, 8, 8192]
    w_qkv_t = w_qkv.rearrange("(ko p) f -> p ko f", p=P)   # [128, 8, 384]
    w_proj_t = w_proj.rearrange("(ko p) f -> p ko f", p=P)  # [128, 8, 128]

    with tile.TileContext(nc) as tc:
        with (
            tc.tile_pool(name="const", bufs=1) as const,
            tc.tile_pool(name="xslab", bufs=2) as xslab_pool,
            tc.tile_pool(name="qk", bufs=2) as qk_pool,
            tc.tile_pool(name="vtok", bufs=2) as v_pool,
            tc.tile_pool(name="pexp", bufs=3) as p_pool,
            tc.tile_pool(name="small", bufs=4) as small_pool,
            tc.tile_pool(name="outsb", bufs=3) as out_pool,
            tc.tile_pool(name="mm_ps", bufs=2, space="PSUM") as mm_ps,
            tc.tile_pool(name="st_ps", bufs=1, space="PSUM") as st_ps,
            tc.tile_pool(name="y_ps", bufs=1, space="PSUM") as y_ps,
            tc.tile_pool(name="bc_ps", bufs=1, space="PSUM") as bc_ps,
            tc.tile_pool(name="dram", bufs=1, space="DRAM") as dram,
        ):
            # ---- constants ----
            wq_sb = const.tile([P, KO, 3 * LC], F32R)      # QKV weights resident
            nc.sync.dma_start(wq_sb[:], w_qkv_t)
            wp_sb = const.tile([P, KO, LC], F32R)          # proj weights resident
            nc.sync.dma_start(wp_sb[:], w_proj_t)
            bqk_sb = const.tile([P, 3], F32)              # per-partition q/k/v bias
            nc.sync.dma_start(bqk_sb[:], b_qkv.rearrange("(c p) -> p c", p=P))
            bp_sb = const.tile([P, 1], F32)               # proj bias (per-partition)
            nc.sync.dma_start(bp_sb[:], b_proj.rearrange("(c p) -> p c", p=P))
            bv_row = const.tile([1, LC], F32)             # v bias as a row
            nc.sync.dma_start(bv_row[:], b_qkv[None, 2 * LC:])
            ones_row = const.tile([1, P], F32)
            nc.vector.memset(ones_row[:], 1.0)
            ones_col = const.tile([P, N_KC, 1], F32)
            nc.vector.memset(ones_col[:], 1.0)
            # broadcast v bias across 128 token partitions via K=1 matmul
            bv_ps = mm_ps.tile([P, LC], F32, tag="mm")
            nc.tensor.matmul(bv_ps[:], ones_row[:, :P], bv_row[:], start=True, stop=True)
            bv_bc = const.tile([P, LC], F32)
            nc.vector.tensor_copy(bv_bc[:], bv_ps[:])

            # diagonal-group causal mask [k within 4*128 rows, q within 512]
            mask_sb = const.tile([P, GRP, QC_W], F32)
            nc.gpsimd.memset(mask_sb[:], 1.0)
            for j in range(GRP):
                # keep where q - k - 128*j >= 0 else 0
                nc.gpsimd.affine_select(
                    out=mask_sb[:, j, :],
                    in_=mask_sb[:, j, :],
                    compare_op=mybir.AluOpType.is_ge,
                    fill=0.0,
                    base=-KC_W * j,
                    pattern=[[1, QC_W]],
                    channel_multiplier=-1,
                )

            # DRAM scratch: local y^T bounce and the AllGather result
            y_bounce = dram.tile([LC, NTOK], F32R)
            yT_full = dram.tile([C, NTOK], F32R, addr_space="Shared")

            # ================= per-batch QKV + attention =================
            for b in range(B):
                qT_b = qk_pool.tile([P, N_QC, QC_W], F32R, tag="qT")   # [128, 4, 512]
                kT_b = qk_pool.tile([P, N_QC, QC_W], F32R, tag="kT")
                v_b = v_pool.tile([P, N_KC, 2 * (HS + 1)], F32R, tag="v")  # [128,16,130]
                nc.vector.tensor_copy(v_b[:, :, HS:HS + 1], ones_col[:])
                nc.vector.tensor_copy(v_b[:, :, 2 * HS + 1:], ones_col[:])

                # ---- QKV for this batch, one 512-token slab at a time ----
                for s in range(N_QC):
                    t0 = b * T + s * QC_W
                    slab = xslab_pool.tile([P, KO, QC_W], F32R, tag="xslab")
                    nc.sync.dma_start(slab[:], xT_t[:, :, t0:t0 + QC_W])

                    # q^T and k^T (feature-major): lhsT = W, rhs = x^T
                    for f, dst in ((0, qT_b), (1, kT_b)):
                        ps = mm_ps.tile([P, QC_W], F32, tag="mm")
                        for k0 in range(KO):
                            nc.tensor.matmul(
                                ps[:],
                                wq_sb[:, k0, f * P:(f + 1) * P],
                                slab[:, k0, :],
                                start=(k0 == 0),
                                stop=(k0 == KO - 1),
                            )
                        nc.vector.tensor_tensor(
                            dst[:, s, :], ps[:],
                            bqk_sb[:, f:f + 1].to_broadcast((P, QC_W)),
                            mybir.AluOpType.add,
                        )

                    # v (token-major): lhsT = x^T, rhs = W_v  -> [128 tok, 128 feat]
                    for t4 in range(QC_W // P):
                        ps = mm_ps.tile([P, P], F32, tag="mm")
                        for k0 in range(KO):
                            nc.tensor.matmul(
                                ps[:],
                                slab[:, k0, t4 * P:(t4 + 1) * P],
                                wq_sb[:, k0, 2 * P:3 * P],
                                start=(k0 == 0),
                                stop=(k0 == KO - 1),
                            )
                        kc = s * (QC_W // P) + t4
                        vps = v_ps_evict = ps  # alias for clarity
                        nc.vector.tensor_tensor(
                            v_b[:, kc, 0:HS], vps[:, 0:HS],
                            bv_bc[:, 0:HS], mybir.AluOpType.add,
                        )
                        nc.vector.tensor_tensor(
                            v_b[:, kc, HS + 1:2 * HS + 1], vps[:, HS:2 * HS],
                            bv_bc[:, HS:2 * HS], mybir.AluOpType.add,
                        )

                # ---- attention for this batch, per local head ----
                for h in range(HPC):
                    hp = h * HS  # partition offset of this head in qT/kT
                    vc = h * (HS + 1)  # column offset of this head in v_b
                    for qc in range(N_QC):
                        yps = y_ps.tile([P, QC_W], F32, tag="y")
                        ngrp = qc + 1
                        for g in range(ngrp):
                            stps = st_ps.tile([P, GRP, QC_W], F32, tag="st")
                            for j in range(GRP):
                                kc = g * GRP + j
                                s_idx, sub = kc // GRP, kc % GRP
                                nc.tensor.matmul(
                                    stps[:, j, :],
                                    kT_b[hp:hp + HS, s_idx,
                                           sub * KC_W:(sub + 1) * KC_W],
                                    qT_b[hp:hp + HS, qc, :],
                                    start=True, stop=True,
                                )
                            pexp = p_pool.tile([P, GRP, QC_W], F32R, tag="p")
                            nc.scalar.activation(
                                pexp[:], stps[:],
                                mybir.ActivationFunctionType.Exp,
                                scale=1.0 / np.sqrt(HS),
                            )
                            if g == qc:  # diagonal group: zero out k > q
                                nc.vector.tensor_tensor(
                                    pexp[:], pexp[:], mask_sb[:],
                                    mybir.AluOpType.mult,
                                )
                            for j in range(GRP):
                                kc = g * GRP + j
                                nc.tensor.matmul(
                                    yps[0:HS + 1, :],
                                    v_b[:, kc, vc:vc + HS + 1],
                                    pexp[:, j, :],
                                    start=(g == 0 and j == 0),
                                    stop=(g == ngrp - 1 and j == GRP - 1),
                                )
                        # normalize: recip of sums row, broadcast via K=1 matmul
                        recip = small_pool.tile([1, QC_W], F32, tag="recip")
                        nc.vector.reciprocal(recip[:], yps[HS:HS + 1, :])
                        bcps = bc_ps.tile([HS, QC_W], F32, tag="bc")
                        nc.tensor.matmul(bcps[:], ones_row[:, :HS], recip[:],
                                         start=True, stop=True)
                        bc_sb = small_pool.tile([HS, QC_W], F32, tag="bc_sb")
                        nc.vector.tensor_copy(bc_sb[:], bcps[:])
                        yout = out_pool.tile([HS, QC_W], F32R, tag="yout")
                        nc.vector.tensor_tensor(
                            yout[:], yps[0:HS, :], bc_sb[:], mybir.AluOpType.mult,
                        )
                        nc.sync.dma_start(
                            y_bounce[hp:hp + HS, b * T + qc * QC_W:
                                     b * T + (qc + 1) * QC_W],
                            yout[:],
                        )

            # ================= AllGather y^T =================
            nc.gpsimd.collective_compute(
                "AllGather",
                mybir.AluOpType.bypass,
                ins=[y_bounce.opt()],
                outs=[yT_full.opt()],
                replica_groups=[list(range(NCORES))],
            )

            # ================= output projection (column slice) =================
            yT_t = yT_full[:].rearrange("(ko p) t -> p ko t", p=P)
            for tn in range(NTOK // QC_W):
                yslab = xslab_pool.tile([P, KO, QC_W], F32R, tag="xslab")
                nc.sync.dma_start(yslab[:], yT_t[:, :, tn * QC_W:(tn + 1) * QC_W])
                ps = mm_ps.tile([P, QC_W], F32, tag="mm")
                for k0 in range(KO):
                    nc.tensor.matmul(
                        ps[:],
                        wp_sb[:, k0, :],
                        yslab[:, k0, :],
                        start=(k0 == 0),
                        stop=(k0 == KO - 1),
                    )
                osb = out_pool.tile([P, QC_W], F32, tag="osb")
                nc.vector.tensor_tensor(
                    osb[:], ps[:], bp_sb[:, 0:1].to_broadcast((P, QC_W)),
                    mybir.AluOpType.add,
                )
                nc.sync.dma_start(out[:, tn * QC_W:(tn + 1) * QC_W], osb[:])

    nc.compile()
    return nc


_NC_CACHE = None


def _get_nc():
    global _NC_CACHE
    if _NC_CACHE is None:
        _NC_CACHE = build_graph()
    return _NC_CACHE


def make_in_maps(x, W_attn, b_attn, W_proj, b_proj):
    x = np.asarray(x, dtype=np.float32)
    W_attn = np.asarray(W_attn, dtype=np.float32)
    b_attn = np.asarray(b_attn, dtype=np.float32)
    W_proj = np.asarray(W_proj, dtype=np.float32)
    b_proj = np.asarray(b_proj, dtype=np.float32)

    xT = np.ascontiguousarray(x.reshape(NTOK, C).T)  # [1024, 8192]
    in_maps = []
    for c in range(NCORES):
        sl = slice(LC * c, LC * (c + 1))
        w_loc = np.ascontiguousarray(np.concatenate(
            [W_attn[:, 0 * C:][:, sl], W_attn[:, 1 * C:][:, sl],
             W_attn[:, 2 * C:][:, sl]], axis=1))  # [1024, 384]
        b_loc = np.ascontiguousarray(np.concatenate(
            [b_attn[0 * C:][sl], b_attn[1 * C:][sl], b_attn[2 * C:][sl]]))
        wp_loc = np.ascontiguousarray(W_proj[:, sl])  # [1024, 128]
        bp_loc = np.ascontiguousarray(b_proj[sl])
        in_maps.append({
            "xT": xT, "w_qkv": w_loc, "b_qkv": b_loc,
            "w_proj": wp_loc, "b_proj": bp_loc,
        })
    return in_maps


def kernel(x, W_attn, b_attn, W_proj, b_proj):
    nc = _get_nc()
    in_maps = make_in_maps(x, W_attn, b_attn, W_proj, b_proj)
    res = bass_utils.run_bass_kernel_spmd(
        nc, in_maps, core_ids=list(range(NCORES)), trace=False,
    )
    outT = np.concatenate([res.results[c]["out"] for c in range(NCORES)], axis=0)
    out = np.ascontiguousarray(outT.T).reshape(B, T, C).astype(np.float32)
    kernel.last_results = res
    return out
